# revision 41
# baseline (speedup 1.0000x reference)
"""CosineGraphAttentionLayer Trainium2 kernel (8-core SPMD, full I/O).

out = softmax(beta * cos_sim(xi, xj) + adj_mask) @ xj,  shapes:
  xi [8192,128] f32, xj [8192,128] f32, adj [8192,8192] int32, beta [1] f32.

Sharding: row-shard xi/adj across 8 cores (1024 rows each), xj replicated;
softmax rows are fully local; host concatenates per-core outputs.

Per-core pipeline (all matmuls fp16 operands, f32 PSUM accumulation).
Iteration granularity = 1024 m-columns (2 chunks paired; 8 iterations):
  - DMA order: xi, xj (4 parts), adj pair-0; adj double-buffered after.
    Head: xi norm pipeline -> uT; xj group 0 -> vT pairs 0-1. Remaining xj
    groups interleave into loop iterations.
  - per iteration: Pool casts adj int32 -> fp16 (one big call); per n-tile:
    mm1 S = uT.T@vT (PSUM f32 [128,1024]), ACT Exp -> fp16 E (SBUF); DVE
    mask-mul E *= adj16 (one [128,8192] 2x call); DVE tensor_scalar row-sum
    accum per tile (4x, [128,1024]); PE-transpose E -> E^T per 128-col
    block, PSUM->SBUF copies split DVE/ACT; mm2: out2T += xj16.T @ E^T
  - finale: out = (out2T / rowsum).T, DMA out.

GPSIMD (Pool) cannot access PSUM on TRN2 — all PSUM traffic is on
DVE/ACT/PE; Pool does the SBUF->SBUF adj cast.
"""
import numpy as np

import concourse.mybir as mybir
import concourse.tile as tile
from concourse import bacc
from concourse.masks import make_identity
from concourse.bass_utils import run_bass_kernel_spmd

dt = mybir.dt
F16 = dt.float16
F32 = dt.float32
AX = mybir.AxisListType.X
MULT = mybir.AluOpType.mult
ADD = mybir.AluOpType.add
Act = mybir.ActivationFunctionType

N_CORES = 8
N, M, D = 8192, 8192, 128
NB = N // N_CORES          # 1024 rows per core
NT = NB // 128             # 8 n-tiles
MTILES = M // 128          # 64
NIT = M // 1024            # 8 iterations of 1024 m-cols
EPS = 1e-07
ACT_COPIES = 35  # of 8 et copies per iteration, how many go to ACT


def _newton_rsqrt(nc, sp, q, n, pfx):
    """y ~= 1/sqrt(q) on [128, n]: reciprocal + sqrt + 2 Newton steps."""
    r = sp.tile([128, n], F32, name=f"{pfx}_r")
    nc.vector.reciprocal(r[:], q[:])
    y = sp.tile([128, n], F32, name=f"{pfx}_y")
    nc.scalar.activation(y[:], r[:], Act.Sqrt)
    t1 = sp.tile([128, n], F32, name=f"{pfx}_t1")
    t3 = sp.tile([128, n], F32, name=f"{pfx}_t3")
    for _ in range(1):
        nc.vector.tensor_mul(t1[:], y[:], y[:])
        nc.vector.tensor_mul(t1[:], t1[:], q[:])
        nc.vector.tensor_scalar(out=t3[:], in0=t1[:], scalar1=-0.5,
                                scalar2=1.5, op0=MULT, op1=ADD)
        nc.vector.tensor_mul(y[:], y[:], t3[:])
    return y


def build(reps=1, nb=NB, m=M):
    nt = nb // 128
    nit = m // 1024
    mtiles = m // 128
    ngrp = mtiles // 16
    nc = bacc.Bacc("TRN2", target_bir_lowering=False, debug=False,
                   num_devices=N_CORES)
    xi = nc.dram_tensor("xi", [nb, D], F32, kind="ExternalInput")
    xj = nc.dram_tensor("xj", [m, D], F32, kind="ExternalInput")
    adj = nc.dram_tensor("adj", [nb, m], dt.int32, kind="ExternalInput")
    beta = nc.dram_tensor("beta", [1], F32, kind="ExternalInput")
    out = nc.dram_tensor("out", [nb, D], F32, kind="ExternalOutput")

    xi_v = xi.ap().rearrange("(t p) d -> p t d", p=128)    # [128, nt, 128]
    xj_v = xj.ap().rearrange("(t p) d -> p t d", p=128)    # [128, mtiles, 128]
    adj_v = adj.ap().rearrange("(t p) m -> p t m", p=128)  # [128, nt, m]
    out_v = out.ap().rearrange("(t p) d -> p t d", p=128)

    from contextlib import ExitStack
    with tile.TileContext(nc) as tc, ExitStack() as ctx:
        pools = {}
        for name, bufs, space in [
            ("const", 1, "SBUF"), ("persist", 1, "SBUF"), ("setup", 1, "SBUF"),
            ("xgp", 2, "SBUF"), ("sqp", 1, "SBUF"), ("v16p", 2, "SBUF"),
            ("vtp", 8, "SBUF"), ("adji", 3, "SBUF"), ("adjf", 3, "SBUF"),
            ("em", 2, "SBUF"), ("ets", 2, "SBUF"), ("fin", 1, "SBUF"),
            ("rsp", 1, "SBUF"), ("scr", 1, "SBUF"),
            ("psS", 2, "PSUM"), ("psET", 2, "PSUM"), ("psO", 1, "PSUM"),
        ]:
            pools[name] = ctx.enter_context(
                tc.tile_pool(name=name, bufs=bufs, space=space))
        cpool, pp, sp = pools["const"], pools["persist"], pools["setup"]
        xgp, sqp, v16p = pools["xgp"], pools["sqp"], pools["v16p"]
        vtp, adjip, adjfp = pools["vtp"], pools["adji"], pools["adjf"]
        emp, etsp, finp = pools["em"], pools["ets"], pools["fin"]
        rspool, scrp = pools["rsp"], pools["scr"]
        psS, psET, psO = pools["psS"], pools["psET"], pools["psO"]
        if True:
            ident16 = cpool.tile([128, 128], F16)
            make_identity(nc, ident16[:])
            ident32 = cpool.tile([128, 128], F32)
            make_identity(nc, ident32[:])

            uT = pp.tile([128, nb], F16)           # normalized beta*xi, transposed
            vTs = [None] * nit
            xj16 = pp.tile([128, mtiles, 128], F16)  # raw xj fp16, natural tiles

            # ---- DMA issue order: xi, xj parts, adj pair 0
            xi_sb = sp.tile([128, nt, 128], F32)
            nc.sync.dma_start(xi_sb[:], xi_v)
            xj_sbs = [None] * ngrp

            def xj_dma(g):
                xg = xgp.tile([128, 16, 128], F32, tag="xg")
                nc.sync.dma_start(xg[:], xj_v[:, g * 16:(g + 1) * 16, :])
                xj_sbs[g] = xg

            xj_dma(0)
            prefetched = {}
            a0 = adjip.tile([128, nt, 512], dt.int32, tag="ai")
            nc.sync.dma_start(a0[:], adj_v[:, :, 0:512])
            prefetched[0] = a0
            xj_dma(1)
            a1 = adjip.tile([128, nt, 512], dt.int32, tag="ai")
            nc.sync.dma_start(a1[:], adj_v[:, :, 512:1024])
            prefetched[1] = a1
            beta_sb = sp.tile([1, 1], F32)
            nc.sync.dma_start(beta_sb[0:1, 0:1], beta.ap()[0:1])
            beta_bc = sp.tile([128, 1], F32)
            nc.gpsimd.partition_broadcast(beta_bc[:], beta_sb[0:1, :])

            # ---- xi-side pipeline: uT ASAP
            qi = sp.tile([128, nt], F32)
            sqi = sqp.tile([128, 16, 128], F32, tag="sq")
            nc.scalar.activation(sqi[:, 0:nt, :], xi_sb[:], Act.Square)
            nc.vector.reduce_sum(qi[:], sqi[:, 0:nt, :], axis=AX)
            yi = _newton_rsqrt(nc, sp, qi, nt, 'i')
            nc.vector.tensor_scalar(out=yi[:], in0=yi[:],
                                    scalar1=beta_bc[:, 0:1], scalar2=None,
                                    op0=MULT)
            u16 = sp.tile([128, nt, 128], F16)
            for t in range(nt):
                nc.vector.tensor_scalar(out=u16[:, t, :], in0=xi_sb[:, t, :],
                                        scalar1=yi[:, t:t + 1], scalar2=None,
                                        op0=MULT)
            uT_ps = psET.tile([128, nb], F16, tag="et")
            for t in range(nt):
                nc.tensor.transpose(uT_ps[:, t * 128:(t + 1) * 128],
                                    u16[:, t, :], ident16[:])
            nc.vector.tensor_copy(uT[:], uT_ps[:])

            # ---- xj-side helpers: per 16-tile group norms -> yjs[g]
            yjs = [None] * ngrp

            def process_group(g):
                sq = sqp.tile([128, 16, 128], F32, tag="sq")
                nc.scalar.activation(sq[:], xj_sbs[g][:], Act.Square)
                qg = sp.tile([128, 16], F32, name=f"qj{g}")
                nc.vector.reduce_sum(qg[:], sq[:], axis=AX)
                # xj16 cast for this group (Pool; ACT is the loop binder)
                nc.gpsimd.tensor_copy(xj16[:, g * 16:(g + 1) * 16, :],
                                      xj_sbs[g][:])
                yjs[g] = _newton_rsqrt(nc, sp, qg, 16, f'j{g}')

            def make_vt(it):
                # vT pair `it` = xj tiles 8*it .. 8*it+7
                g, o = it // 2, (it % 2) * 8
                v16 = v16p.tile([128, 8, 128], F16, tag="v16")
                for k in range(8):
                    nc.vector.tensor_scalar(
                        out=v16[:, k, :], in0=xj_sbs[g][:, o + k, :],
                        scalar1=yjs[g][:, o + k:o + k + 1],
                        scalar2=None, op0=MULT)
                tp = psET.tile([128, 1024], F16, tag="et")
                for k in range(8):
                    nc.tensor.transpose(tp[:, k * 128:(k + 1) * 128],
                                        v16[:, k, :], ident16[:])
                vt = vtp.tile([128, 1024], F16, tag="vt")
                nc.vector.tensor_copy(vt[:], tp[:])
                vTs[it] = vt

            process_group(0)
            make_vt(0)
            make_vt(1)

            # ---------------- main loop ----------------
            for rep in range(reps):
                rs_parts = rspool.tile([128, nt * nit], F32, tag="rsp")
                out2T = psO.tile([128, nb], F32, tag="o2")  # [d, n] accum
                for it in range(nit):
                    if rep == 0 and it % 2 == 0 and it // 2 + 2 < ngrp:
                        xj_dma(it // 2 + 2)
                    if rep == 0 and it % 2 == 0 and it // 2 + 1 < ngrp:
                        process_group(it // 2 + 1)
                    if rep == 0 and it + 2 < nit:
                        make_vt(it + 2)

                    adj16s = []
                    for c in (2 * it, 2 * it + 1):
                        if c in prefetched and rep == 0:
                            adj_c = prefetched.pop(c)
                        else:
                            adj_c = adjip.tile([128, nt, 512], dt.int32,
                                               tag="ai")
                            nc.sync.dma_start(
                                adj_c[:],
                                adj_v[:, :, c * 512:(c + 1) * 512])
                        a16 = adjfp.tile([128, nt, 512], F16, tag="af")
                        nc.gpsimd.tensor_copy(a16[:], adj_c[:])
                        adj16s.append(a16)

                    em_all = emp.tile([128, nt, 1024], F16, tag="em")
                    for t in range(nt):
                        s_ps = psS.tile([128, 1024], F32, tag="s")
                        # one 512-col matmul per PSUM bank (matmul output
                        # must not span banks); one Exp over both
                        for hh in range(2):
                            nc.tensor.matmul(s_ps[:, hh * 512:(hh + 1) * 512],
                                             uT[:, t * 128:(t + 1) * 128],
                                             vTs[it][:, hh * 512:(hh + 1) * 512],
                                             start=True, stop=True)
                        nc.scalar.activation(em_all[:, t, :], s_ps[:], Act.Exp)
                    # mask: one DVE multiply per 512-half (fp16, 2x mode)
                    for hh in range(2):
                        nc.vector.tensor_mul(
                            em_all[:, :, hh * 512:(hh + 1) * 512],
                            em_all[:, :, hh * 512:(hh + 1) * 512],
                            adj16s[hh][:])
                    for t in range(nt):
                        # row-sum accum; `out` is a throwaway scratch so the
                        # PE transposes don't wait on these writes
                        scr = scrp.tile([128, 1024], F16, tag="scr")
                        nc.vector.tensor_scalar(
                            out=scr[:], in0=em_all[:, t, :],
                            scalar1=1.0, scalar2=0.0, op0=MULT, op1=ADD,
                            accum_out=rs_parts[:, t * nit + it:t * nit + it + 1])
                    for j in range(8):
                        et_ps = psET.tile([128, nb], F16, tag="et")
                        for t in range(nt):
                            nc.tensor.transpose(et_ps[:, t * 128:(t + 1) * 128],
                                                em_all[:, t, j * 128:(j + 1) * 128],
                                                ident16[:])
                        et_sb = etsp.tile([128, nb], F16, tag="ets")
                        # PSUM->SBUF copy split DVE/ACT; ACT share set by
                        # ACT_COPIES (ACT Exp<->Copy table switches are free
                        # in sim but may cost ~1.3us each on HW)
                        if it >= nit - 2:
                            # tail: ACT idles after its last Exps while DVE
                            # drains -- give ACT most of the final copies
                            act_copy = (j != 0) if it == nit - 1 else (j % 2 == 1)
                        else:
                            act_copy = j in (1, 3, 5) or (j == 7 and it % 2 == 0)
                        if act_copy:
                            nc.scalar.activation(et_sb[:], et_ps[:], Act.Copy)
                        else:
                            nc.vector.tensor_copy(et_sb[:], et_ps[:])
                        mt = it * 8 + j
                        hw_ = min(512, nb)
                        for h in range(nb // hw_):
                            nc.tensor.matmul(out2T[:, h * hw_:(h + 1) * hw_],
                                             xj16[:, mt, :],
                                             et_sb[:, h * hw_:(h + 1) * hw_],
                                             start=(mt == 0),
                                             stop=(mt == mtiles - 1))

                # ---------------- finale ----------------
                rs = finp.tile([128, nt], F32, tag="rs")
                nc.vector.reduce_sum(
                    rs[:], rs_parts[:].rearrange("p (t c) -> p t c", t=nt),
                    axis=AX)
                rrs = finp.tile([128, nt], F32, tag="rrs")
                nc.vector.reciprocal(rrs[:], rs[:])
                o2_sb = finp.tile([128, nb], F32, tag="o2sb")
                nc.vector.tensor_copy(o2_sb[:], out2T[:])
                out_sb = finp.tile([128, nt, 128], F32, tag="osb")
                for t in range(nt):
                    ot_ps = psS.tile([128, 128], F32, tag="s")
                    nc.tensor.transpose(ot_ps[:], o2_sb[:, t * 128:(t + 1) * 128],
                                        ident32[:])
                    nc.vector.tensor_scalar(out=out_sb[:, t, :], in0=ot_ps[:],
                                            scalar1=rrs[:, t:t + 1],
                                            scalar2=None, op0=MULT)
                nc.sync.dma_start(out_v, out_sb[:])
    nc.compile()
    return nc


_NC_CACHE = {}


def _get_nc(reps=1):
    if reps not in _NC_CACHE:
        _NC_CACHE[reps] = build(reps=reps)
    return _NC_CACHE[reps]


def kernel(xi, xj, adj, beta):
    xi = np.ascontiguousarray(np.asarray(xi, dtype=np.float32))
    xj = np.ascontiguousarray(np.asarray(xj, dtype=np.float32))
    adj = np.ascontiguousarray(np.asarray(adj, dtype=np.int32))
    beta = np.ascontiguousarray(np.asarray(beta, dtype=np.float32))
    nc = _get_nc(reps=1)
    in_maps = []
    for c in range(N_CORES):
        sl = slice(c * NB, (c + 1) * NB)
        in_maps.append({
            "xi": np.ascontiguousarray(xi[sl]),
            "xj": xj,
            "adj": np.ascontiguousarray(adj[sl]),
            "beta": beta,
        })
    res = run_bass_kernel_spmd(nc, in_maps, core_ids=list(range(N_CORES)))
    return np.concatenate([res.results[c]["out"] for c in range(N_CORES)], axis=0)
